# revision 5
# baseline (speedup 1.0000x reference)
"""BipartiteSAGEConv on 8 Trainium2 NeuronCores.

out = normalize(mean_{dst}(x[src]) @ W_l + b_l + x @ W_r)

Strategy:
- Host: sort edges by destination node, shard destination-node ranges across
  the 8 cores (each core owns 12500 contiguous nodes and all edges pointing
  into them -> no cross-core reduction needed). Per 128-node tile, edges are
  grouped by src bank (4 banks of 25024 rows, since dma_gather indices are
  int16) and packed into KB chunks of 128 per bank (padded; padding edges
  carry dstrel=-1 so the one-hot kills them). The per-edge weight
  w = 1/max(deg(dst),1) is folded into the one-hot so the PSUM accumulation
  yields the mean directly.
- Device (SPMD, identical program on all 8 cores):
  * dma_gather (Ant SWDGE gather, int16 idx) of x[src] rows per (tile, bank)
  * DVE builds the weighted one-hot: (iota == dstrel) * w  (one fused op)
  * PE accumulates meanT[f, n] += msg[e, f].T @ onehot[e, n] over chunks
  * PE: out[n, fo] = meanT.T @ W_l + xT.T @ W_r + ones x b_l (one PSUM group)
  * ACT Square+accum -> row sum of squares; sqrt; clamp; DVE reciprocal;
    scale rows; DMA out.
"""

import numpy as np

N_NODES = 100000
D = 128
N_CORES = 8
NODES_PER_CORE = N_NODES // N_CORES  # 12500
P = 128
TILES_PER_CORE = (NODES_PER_CORE + P - 1) // P  # 98
NODE_PAD = TILES_PER_CORE * P  # 12544
X_PAD_ROWS = 100096  # 782 * 128; >= 7*12500 + 12544
BANK = X_PAD_ROWS // 4  # 25024 rows per gather bank (< 32768 int16 limit)
NBANKS = 4

_program_cache = {}

# test harness hooks
TRACE = False
LAST = {}


def _build_program(KB: int):
    """Build + compile the SPMD Bass program; KB = edge chunks per (tile, bank)."""
    import concourse.bass as bass
    import concourse.tile as tile
    from concourse import bacc, mybir
    from concourse.masks import make_identity

    f32 = mybir.dt.float32
    i16 = mybir.dt.int16
    KT = NBANKS * KB  # chunk slots per tile
    NIDX = KB * P  # indices per gather
    IW = NIDX // 16  # idx columns per (tile, bank)

    nc = bacc.Bacc(
        "TRN2", target_bir_lowering=False, debug=False, num_devices=N_CORES
    )

    xpad = nc.dram_tensor("xpad", [X_PAD_ROWS, D], f32, kind="ExternalInput")
    xchunk = nc.dram_tensor("xchunk", [NODE_PAD, D], f32, kind="ExternalInput")
    gidx = nc.dram_tensor(
        "gidx", [P, TILES_PER_CORE, NBANKS, IW], i16, kind="ExternalInput"
    )
    dstrel = nc.dram_tensor("dstrel", [P, TILES_PER_CORE, KT], f32, kind="ExternalInput")
    wgt = nc.dram_tensor("wgt", [P, TILES_PER_CORE, KT], f32, kind="ExternalInput")
    wl = nc.dram_tensor("wl", [D, D], f32, kind="ExternalInput")
    wr = nc.dram_tensor("wr", [D, D], f32, kind="ExternalInput")
    bl = nc.dram_tensor("bl", [1, D], f32, kind="ExternalInput")
    out = nc.dram_tensor("out", [NODE_PAD, D], f32, kind="ExternalOutput")

    with tile.TileContext(nc) as tc:
        with (
            tc.tile_pool(name="const", bufs=1) as const_pool,
            tc.tile_pool(name="meta", bufs=1) as meta_pool,
            tc.tile_pool(name="msg", bufs=3 * NBANKS) as msg_pool,
            tc.tile_pool(name="oh", bufs=6) as oh_pool,
            tc.tile_pool(name="xt", bufs=3) as xt_pool,
            tc.tile_pool(name="ep", bufs=3) as ep_pool,
            tc.tile_pool(name="ps_mean", bufs=2, space="PSUM") as ps_mean_pool,
            tc.tile_pool(name="ps_xt", bufs=2, space="PSUM") as ps_xt_pool,
            tc.tile_pool(name="ps_out", bufs=2, space="PSUM") as ps_out_pool,
        ):
            # ---- constants / weights / metadata (loaded once) ----
            iota_i = const_pool.tile([P, P], mybir.dt.int32)
            nc.gpsimd.iota(iota_i[:], pattern=[[1, P]], base=0, channel_multiplier=0)
            iota_f = const_pool.tile([P, P], f32)
            nc.vector.tensor_copy(iota_f[:], iota_i[:])

            identity = const_pool.tile([P, P], f32)
            make_identity(nc, identity[:])

            wl_sb = const_pool.tile([D, D], f32)
            nc.sync.dma_start(wl_sb[:], wl[:])
            wr_sb = const_pool.tile([D, D], f32)
            nc.sync.dma_start(wr_sb[:], wr[:])
            bl_sb = const_pool.tile([1, D], f32)
            nc.sync.dma_start(bl_sb[:], bl[:])
            ones1 = const_pool.tile([1, D], f32)
            nc.vector.memset(ones1[:], 1.0)

            idx_all = meta_pool.tile([P, TILES_PER_CORE, NBANKS, IW], i16)
            nc.sync.dma_start(idx_all[:], gidx[:])
            dst_all = meta_pool.tile([P, TILES_PER_CORE, KT], f32)
            nc.sync.dma_start(dst_all[:], dstrel[:])
            w_all = meta_pool.tile([P, TILES_PER_CORE, KT], f32)
            nc.sync.dma_start(w_all[:], wgt[:])

            # ---- main loop over node tiles ----
            for t in range(TILES_PER_CORE):
                # gather x[src] per bank: msg position (p, j) <- edge j*128+p
                msgs = []
                for b in range(NBANKS):
                    msg = msg_pool.tile([P, KB, D], f32, tag="msg")
                    nc.gpsimd.dma_gather(
                        out_ap=msg[:],
                        in_ap=xpad[b * BANK : (b + 1) * BANK, :],
                        idxs_ap=idx_all[:, t, b, :],
                        num_idxs=NIDX,
                        num_idxs_reg=NIDX,
                        elem_size=D,
                    )
                    msgs.append(msg)

                # root path: x tile, transposed via PE
                x_sb = xt_pool.tile([P, D], f32, tag="x_in")
                nc.sync.dma_start(x_sb[:], xchunk[t * P : (t + 1) * P, :])
                ps_xt = ps_xt_pool.tile([P, P], f32)
                nc.tensor.transpose(out=ps_xt[:], in_=x_sb[:], identity=identity[:])
                xT_sb = xt_pool.tile([P, D], f32, tag="x_t")
                nc.scalar.copy(xT_sb[:], ps_xt[:])

                # mean aggregation: meanT[f, n] accumulated over chunk slots
                ps_mean = ps_mean_pool.tile([P, P], f32)
                for s in range(KT):
                    b, j = divmod(s, KB)
                    oh = oh_pool.tile([P, P], f32)
                    nc.vector.tensor_scalar(
                        oh[:],
                        iota_f[:],
                        dst_all[:, t, s : s + 1],
                        w_all[:, t, s : s + 1],
                        mybir.AluOpType.is_equal,
                        mybir.AluOpType.mult,
                    )
                    nc.tensor.matmul(
                        out=ps_mean[:],
                        lhsT=msgs[b][:, j, :],
                        rhs=oh[:],
                        start=(s == 0),
                        stop=(s == KT - 1),
                    )
                meanT_sb = ep_pool.tile([P, P], f32, tag="meanT")
                nc.scalar.copy(meanT_sb[:], ps_mean[:])

                # linear: out[n, fo] = meanT.T @ W_l + xT.T @ W_r + ones x b_l
                ps_o = ps_out_pool.tile([P, P], f32)
                nc.tensor.matmul(
                    out=ps_o[:], lhsT=meanT_sb[:], rhs=wl_sb[:], start=True, stop=False
                )
                nc.tensor.matmul(
                    out=ps_o[:], lhsT=xT_sb[:], rhs=wr_sb[:], start=False, stop=False
                )
                nc.tensor.matmul(
                    out=ps_o[:], lhsT=ones1[:], rhs=bl_sb[:], start=False, stop=True
                )

                # row-wise L2 normalize: out / max(||out||, 1e-12)
                sq_scr = ep_pool.tile([P, P], f32, tag="sq")
                ss = ep_pool.tile([P, 1], f32, tag="ss")
                nc.scalar.activation(
                    sq_scr[:],
                    ps_o[:],
                    mybir.ActivationFunctionType.Square,
                    accum_out=ss[:],
                )
                nrm = ep_pool.tile([P, 1], f32, tag="nrm")
                nc.scalar.sqrt(nrm[:], ss[:])
                nrmc = ep_pool.tile([P, 1], f32, tag="nrmc")
                nc.vector.tensor_scalar_max(nrmc[:], nrm[:], 1e-12)
                rn = ep_pool.tile([P, 1], f32, tag="rn")
                nc.vector.reciprocal(rn[:], nrmc[:])

                out_sb = ep_pool.tile([P, P], f32, tag="out")
                nc.vector.tensor_scalar(
                    out_sb[:],
                    ps_o[:],
                    rn[:, :1],
                    None,
                    mybir.AluOpType.mult,
                )
                nc.sync.dma_start(out[t * P : (t + 1) * P, :], out_sb[:])

    nc.compile()
    return nc


def _prepare(x, edge_index):
    """Host-side sharding: sort by dst, group per (tile, bank), pack chunks."""
    src = np.ascontiguousarray(edge_index[0]).astype(np.int64)
    dst = np.ascontiguousarray(edge_index[1]).astype(np.int64)

    cnt = np.bincount(dst, minlength=N_NODES)
    w_node = (1.0 / np.maximum(cnt, 1)).astype(np.float32)

    order = np.argsort(dst, kind="stable")
    src_s = src[order]
    dst_s = dst[order]

    # per-core edge ranges and per (core,tile,bank) grouping
    per_core = []
    KB = 1
    for c in range(N_CORES):
        base = c * NODES_PER_CORE
        lo = np.searchsorted(dst_s, base)
        hi = np.searchsorted(dst_s, base + NODES_PER_CORE)
        s_c = src_s[lo:hi]
        d_c = dst_s[lo:hi] - base
        t_c = d_c // P
        b_c = s_c // BANK
        key = (t_c * NBANKS + b_c).astype(np.int64)
        ordc = np.argsort(key, kind="stable")
        s_c, d_c, key = s_c[ordc], d_c[ordc], key[ordc]
        counts = np.bincount(key, minlength=TILES_PER_CORE * NBANKS)
        KB = max(KB, int(np.ceil(counts.max() / P)))
        per_core.append((s_c, d_c, counts))

    KT = NBANKS * KB
    NIDX = KB * P
    IW = NIDX // 16

    gidx = np.zeros((N_CORES, P, TILES_PER_CORE, NBANKS, IW), np.int16)
    dstrel = np.full((N_CORES, P, TILES_PER_CORE, KT), -1.0, np.float32)
    wgt = np.zeros((N_CORES, P, TILES_PER_CORE, KT), np.float32)

    prow = np.arange(P) % 16
    scol = np.arange(IW) * 16
    for c in range(N_CORES):
        s_c, d_c, counts = per_core[c]
        starts = np.concatenate([[0], np.cumsum(counts)])
        for t in range(TILES_PER_CORE):
            for b in range(NBANKS):
                g = t * NBANKS + b
                n = counts[g]
                if n == 0:
                    continue
                lo = starts[g]
                sv = s_c[lo : lo + n] - b * BANK
                dv = (d_c[lo : lo + n] - t * P).astype(np.float32)
                wv = w_node[d_c[lo : lo + n] + c * NODES_PER_CORE]
                i_pad = np.zeros(NIDX, np.int16)
                i_pad[:n] = sv.astype(np.int16)
                d_pad = np.full(NIDX, -1.0, np.float32)
                d_pad[:n] = dv
                w_pad = np.zeros(NIDX, np.float32)
                w_pad[:n] = wv
                # idx position i lives at [i % 16, i // 16], replicated %16
                gidx[c, :, t, b, :] = i_pad[scol[None, :] + prow[:, None]]
                # chunk slot s=b*KB+j, lane p <- edge j*128+p
                dstrel[c, :, t, b * KB : (b + 1) * KB] = d_pad.reshape(KB, P).T
                wgt[c, :, t, b * KB : (b + 1) * KB] = w_pad.reshape(KB, P).T

    return gidx, dstrel, wgt, KB


def kernel(x, edge_index, W_l, b_l, W_r):
    from concourse.bass_utils import run_bass_kernel_spmd

    x = np.ascontiguousarray(np.asarray(x, dtype=np.float32))
    W_l = np.ascontiguousarray(np.asarray(W_l, dtype=np.float32))
    W_r = np.ascontiguousarray(np.asarray(W_r, dtype=np.float32))
    b_l = np.ascontiguousarray(np.asarray(b_l, dtype=np.float32)).reshape(1, D)

    gidx, dstrel, wgt, KB = _prepare(x, np.asarray(edge_index))

    xpad = np.zeros((X_PAD_ROWS, D), np.float32)
    xpad[:N_NODES] = x

    if KB not in _program_cache:
        _program_cache[KB] = _build_program(KB)
    nc = _program_cache[KB]

    in_maps = []
    for c in range(N_CORES):
        base = c * NODES_PER_CORE
        in_maps.append(
            {
                "xpad": xpad,
                "xchunk": xpad[base : base + NODE_PAD],
                "gidx": gidx[c],
                "dstrel": dstrel[c],
                "wgt": wgt[c],
                "wl": W_l,
                "wr": W_r,
                "bl": b_l,
            }
        )

    LAST["nc"] = nc
    LAST["in_maps"] = in_maps
    r = run_bass_kernel_spmd(nc, in_maps, list(range(N_CORES)), trace=TRACE)
    LAST["exec_time_ns"] = r.exec_time_ns
    res = r.results
    out = np.concatenate(
        [res[c]["out"][:NODES_PER_CORE] for c in range(N_CORES)], axis=0
    )
    return out
